# revision 30
# baseline (speedup 1.0000x reference)
"""Decode-step multi-head attention with KV cache (DeepSpeed-inference style).

Full shapes (hardcoded per problem spec):
  query/key/value: [16, 1, 2048] f32
  key_cache/value_cache: [16, 16, 4096, 128] f32
  cache_len: scalar int (2048)
Output: [16, 1, 2048] f32

Strategy: data-parallel over batch across 8 NeuronCores (2 batches/core =
32 (batch, head) pairs per core). Per pair, the core streams the K and V
cache slices ([cache_len, 128] each) from HBM, computes scores with
multiply+reduce on VectorE (K stays in its natural [k, d] layout), exp via
ScalarE (with fused row-sum for the softmax denominator), and aggregates
V with TensorE matmuls: the probability column is the (tiny) stationary
weight and V streams through as the f32 moving operand, so V needs no
dtype cast anywhere and each pair's output lands as a PSUM row
[pair, head_dim]. The new token's contribution is one extra 1x1-weight
matmul at the start of each pair's accumulation group.

The kernel is bound by the 16 SDMA engines' HBM read side (~25 GB/s per
engine; K+V cache = 64MiB/core). Both K and V ride the sync (SP) HWDGE
queue in f32. No SWDGE: its SBUF descriptor rings sit on partition lines
whose AXI ports also serve SDMA engine 15, throttling that engine ~17%
below its peers - and since every DMA stripes over all 16 engines and
completion waits on the slowest, SWDGE ring traffic paced the whole
stream (~338 GB/s vs ~400 achievable on pure HWDGE).

- ScalarE casts each K tile to f16 so the score multiply+reduce runs at
  16-bit VectorE throughput - both engines keep slack against the DMA
  stream even on cores with slow engine clocks.
- Small setup loads ride the scalar (ACT) HWDGE queue; queries are
  replicated across partitions with a PE outer product (ones x q_row),
  not a 2MB broadcast DMA.
- The last pair streams K fully before V, computes all scores while V
  streams, and emits its PV matmuls per V-half, so after the final
  packet only a half's matmuls + normalize + 16KB out DMA remain.
"""

import functools
import os
from contextlib import ExitStack

import numpy as np

import concourse.bacc as bacc
import concourse.bass as bass
import concourse.mybir as mybir
import concourse.tile as tile
from concourse import bass_utils

N_CORES = 8
P = 128  # partitions

# test.py hooks: set TRACE=True before calling kernel() to collect a profile.
TRACE = False
TRACE_KWARGS = {}
LAST_RESULTS = None


def _build_program(bl: int, n_heads: int, max_seq: int, hd: int, cache_len: int):
    """Build + compile the per-core program. bl = local batch count."""
    npairs = bl * n_heads
    assert hd == P
    nch = cache_len // P          # full 128-row chunks of the cache
    rem = cache_len - nch * P     # remainder rows
    assert rem == 0, "cache_len % 128 != 0 not needed for this problem"
    ncht = nch
    sm_scale = 1.0 / float(np.sqrt(hd))
    # The last pairs' K tiles load early (on the other ring) so their score
    # work finishes during the stream; their V tiles are the final DMAs, so
    # only one pair's PV matmul train + normalize trails the last packet.
    N_EARLY_K = 2 if npairs >= 8 else 0   # K for the last two pairs, early
    N_LATE_V = 3 if npairs >= 8 else 0    # V for the last three pairs, late

    nc = bacc.Bacc("TRN2", target_bir_lowering=False, debug=False)
    f32 = mybir.dt.float32
    f16 = mybir.dt.float16

    kc = nc.dram_tensor("kc", [bl, n_heads, max_seq, hd], f32, kind="ExternalInput").ap()
    vc = nc.dram_tensor("vc", [bl, n_heads, max_seq, hd], f32, kind="ExternalInput").ap()
    q = nc.dram_tensor("q", [npairs, hd], f32, kind="ExternalInput").ap()
    kn = nc.dram_tensor("kn", [npairs, hd], f32, kind="ExternalInput").ap()
    vn = nc.dram_tensor("vn", [npairs, hd], f32, kind="ExternalInput").ap()
    ident = nc.dram_tensor("ident", [P, P], f32, kind="ExternalInput").ap()
    out = nc.dram_tensor("out", [npairs, hd], f32, kind="ExternalOutput").ap()

    with tile.TileContext(nc) as tc, ExitStack() as ctx:
        singles = ctx.enter_context(tc.tile_pool(name="singles", bufs=1))
        # q_row is only read during setup (the q broadcast); final_row's
        # first write comes long after - share one 64KB row slot
        rows = ctx.enter_context(tc.tile_pool(name="rows", bufs=1))
        kbufs = int(os.environ.get("KBUFS", "8"))
        kpool = ctx.enter_context(tc.tile_pool(name="kpool", bufs=kbufs - 1))
        vpool = ctx.enter_context(tc.tile_pool(name="vpool", bufs=kbufs))
        k16pool = ctx.enter_context(tc.tile_pool(name="k16pool", bufs=2))
        ppool = ctx.enter_context(tc.tile_pool(name="ppool", bufs=2))
        stats = ctx.enter_context(tc.tile_pool(name="stats", bufs=4))
        psum_o = ctx.enter_context(tc.tile_pool(name="psum_o", bufs=3, space="PSUM"))
        psum_q = ctx.enter_context(tc.tile_pool(name="psum_q", bufs=2, space="PSUM"))
        psum_1 = ctx.enter_context(tc.tile_pool(name="psum_1", bufs=1, space="PSUM"))

        # K rides the sync (SP) HWDGE ring, V the scalar (ACT) HWDGE ring.
        # Two independent rings keep every SDMA engine at its ~25 GB/s read
        # rate (a single ring serialized packets and dropped all engines to
        # ~22 GB/s). No SWDGE anywhere: its SBUF descriptor rings throttle
        # SDMA engine 15 ~17% below its peers, and since every DMA stripes
        # over all 16 engines and completion waits on the slowest, that
        # paced the whole stream. To balance ring bytes with the reordered
        # tail (K of the last N_EARLY_K pairs early on the scalar ring, V of
        # the last N_LATE_V pairs last on the sync ring), the rings swap
        # those tiles.
        def kslc(p):
            b, h = divmod(p, n_heads)
            return kc[b, h, 0 : nch * P, :].rearrange("(p c) d -> p c d", c=nch)

        def vslc(p):
            b, h = divmod(p, n_heads)
            return vc[b, h, 0 : nch * P, :].rearrange("(p c) d -> p c d", c=nch)

        def emit_k(p, engine=None, pool=None):
            kt = (pool or kpool).tile([P, ncht, hd], f32, tag=f"kt{p}" if pool else "kt")
            (engine or nc.sync).dma_start(out=kt, in_=kslc(p))
            return kt

        def emit_v(p, engine=None):
            vt = vpool.tile([P, ncht, hd], f32, tag="vt")
            (engine or nc.scalar).dma_start(out=vt, in_=vslc(p))
            return vt

        # issue the first pairs' K loads before any setup traffic so the
        # sync ring's first instruction is a K DMA
        PRELOAD = min(3, npairs)
        kts = {p: emit_k(p) for p in range(PRELOAD)}
        vts = {}

        ones_col = singles.tile([P, 1], f32)
        nc.vector.memset(ones_col, 1.0)

        # small setup loads lead the scalar (ACT) HWDGE ring (tiny, so they
        # barely delay the V stream)
        def flat_row(t):
            return bass.AP(tensor=t.tensor, offset=t.offset, ap=[[0, 1], [1, npairs * hd]])

        q_row = rows.tile([1, npairs * hd], f32, tag="row")
        nc.scalar.dma_start(out=q_row, in_=flat_row(q))
        vn_row = singles.tile([1, npairs * hd], f32)
        nc.scalar.dma_start(out=vn_row, in_=flat_row(vn))
        kn_all = singles.tile([npairs, hd], f32)
        nc.scalar.dma_start(out=kn_all, in_=kn)
        q_all = singles.tile([npairs, hd], f32)
        nc.scalar.dma_start(out=q_all, in_=q)
        ident_sb = singles.tile([P, P], f32)
        nc.scalar.dma_start(out=ident_sb, in_=ident)

        # V preloads, then the last pairs' K tiles (pinned in their own
        # pool slots), all near the head of the scalar ring
        for p in range(PRELOAD):
            vts[p] = emit_v(p)
        early_k = ctx.enter_context(tc.tile_pool(name="early_k", bufs=1))
        for p in range(npairs - N_EARLY_K, npairs):
            kts[p] = emit_k(p, engine=nc.scalar, pool=early_k)

        # all queries broadcast to every partition, once, as a PE outer
        # product ones[1,128] x q_row[1,*] (not DMA: a 2MB broadcast DMA
        # costs ~6.5us of DMA engine time). f16 replicas feed the 16-bit
        # score path.
        ones_row = singles.tile([1, P], f32)
        nc.vector.memset(ones_row, 1.0)
        q_all_b = singles.tile([P, npairs, hd], f16)
        GPAIRS = 4  # pairs per chunk; 4*hd f32 = one 2KB PSUM bank
        ngrp = npairs // GPAIRS
        for g in range(ngrp):
            qb_ps = psum_q.tile([P, GPAIRS, hd], f32, tag="qb")
            qb_2d = bass.AP(
                tensor=qb_ps.tensor,
                offset=qb_ps.offset,
                ap=[qb_ps.ap[0], [1, GPAIRS * hd]],
            )
            nc.tensor.matmul(
                qb_2d,
                lhsT=ones_row,
                rhs=q_row[0:1, g * GPAIRS * hd : (g + 1) * GPAIRS * hd],
                start=True,
                stop=True,
            )
            nc.scalar.copy(q_all_b[:, g * GPAIRS : (g + 1) * GPAIRS, :], qb_ps)

        # Softmax denominators, one column per pair (partition 0).
        lrow = psum_1.tile([1, npairs], f32, tag="l")
        # Normalized output rows, all on partition 0, emitted with one DMA
        # (reuses q_row's slot - setup reads are done before pair 0 ends).
        final_row = rows.tile([1, npairs * hd], f32, tag="row")

        # ---- new-token scores, batched over all pairs, ending in a
        # partition-0 row p_newT so each pair's PV group can start with a
        # 1x1-weight matmul (PE requires base partition 0/32/64) ----
        prod_new = singles.tile([npairs, hd], f32)
        nc.vector.tensor_mul(prod_new, kn_all, q_all)
        s_new = singles.tile([npairs, 1], f32)
        nc.vector.reduce_sum(s_new, prod_new, axis=mybir.AxisListType.X)
        s_newT_ps = psum_1.tile([1, npairs], f32, tag="snT")
        nc.tensor.matmul(
            s_newT_ps, lhsT=s_new, rhs=ident_sb[:npairs, :npairs], start=True, stop=True
        )
        p_newT = singles.tile([1, npairs], f32)
        nc.scalar.activation(
            out=p_newT,
            in_=s_newT_ps,
            func=mybir.ActivationFunctionType.Exp,
            scale=sm_scale,
        )

        def bcast(ap2d, nb):
            return bass.AP(
                tensor=ap2d.tensor,
                offset=ap2d.offset,
                ap=[ap2d.ap[0], [0, nb], ap2d.ap[1]],
            )

        # lrow starts as p_newT (one 1x1 matmul); each pair then accumulates
        # its denominator into its column (start=False), so no epilogue
        # p_new add is needed
        nc.tensor.matmul(
            lrow, lhsT=ones_col[0:1, 0:1], rhs=p_newT, start=True, stop=True
        )

        def emit_scores(p, kt, cs, nq, tag_sfx=""):
            """Score block for chunks cs (len nq) of pair p: f16 cast ->
            mul -> pairwise folds -> reduce -> exp (+denominator accum).
            Returns the f32 probability tile [P, nq]."""
            kt16 = k16pool.tile([P, nq, hd], f16, tag="kt16" + tag_sfx)
            nc.scalar.copy(kt16, kt[:, cs, :])
            prod = ppool.tile([P, nq, hd], f16, tag="prod" + tag_sfx)
            nc.vector.tensor_mul(prod, kt16, bcast(q_all_b[:, p, :], nq))
            fold1 = ppool.tile([P, nq, hd // 2], f16, tag="f1" + tag_sfx)
            nc.vector.tensor_add(fold1, prod[:, :, : hd // 2], prod[:, :, hd // 2 :])
            fold2 = ppool.tile([P, nq, hd // 4], f16, tag="f2" + tag_sfx)
            nc.vector.tensor_add(fold2, fold1[:, :, : hd // 4], fold1[:, :, hd // 4 :])
            s_tile = stats.tile([P, nq], f32, tag="s" + tag_sfx)
            nc.vector.reduce_sum(s_tile, fold2, axis=mybir.AxisListType.X)
            p_tile = stats.tile([P, nq], f32, tag="p" + tag_sfx)
            l_part = stats.tile([P, 1], f32, tag="l" + tag_sfx)
            nc.scalar.activation(
                out=p_tile,
                in_=s_tile,
                func=mybir.ActivationFunctionType.Exp,
                scale=sm_scale,
                accum_out=l_part,
            )
            return p_tile, l_part

        def emit_l_accum(p, l_part, stop):
            nc.tensor.matmul(
                lrow[0:1, p : p + 1], lhsT=ones_col, rhs=l_part, start=False, stop=stop
            )

        def emit_pv_start(p, acc_p):
            # start the pair's PV accumulation group with the new token:
            # acc_p = p_newT[p] * vn_row[p*hd:(p+1)*hd]  (all partition 0)
            nc.tensor.matmul(
                acc_p,
                lhsT=p_newT[0:1, p : p + 1],
                rhs=vn_row[0:1, p * hd : (p + 1) * hd],
                start=True,
                stop=False,
            )

        def emit_pv(vt, acc_p, p_tile, vt_c0, pt_c0, cn, stop_at_end):
            # probability column stationary, V f32 moving: no V cast needed
            for i in range(cn):
                nc.tensor.matmul(
                    acc_p,
                    lhsT=p_tile[:, pt_c0 + i : pt_c0 + i + 1],
                    rhs=vt[:, vt_c0 + i, :],
                    start=False,
                    stop=(stop_at_end and i == cn - 1),
                )

        def emit_normalize(p, acc_p):
            # per-pair normalize straight out of PSUM into the output row
            # buffer (runs mid-stream for every pair but the last)
            recip_p = stats.tile([1, 1], f32, tag="r")
            nc.vector.reciprocal(recip_p, lrow[0:1, p : p + 1])
            nc.scalar.mul(
                final_row[0:1, p * hd : (p + 1) * hd], acc_p, mul=recip_p
            )

        # Ring layout: sync = [K0..K(n-3), V(n-3), V(n-2), V(n-1)],
        # scalar = [setup, V0..V2, K(n-2), K(n-1), V3..V(n-4)]. The rings
        # stay byte-balanced, the last pairs' K lands early (so their score
        # work drains during the stream), and the final packets are the
        # last pairs' V - after which only their PV matmul trains + the
        # per-pair normalize + one 16KB out DMA remain.
        for p in range(npairs):
            if p not in kts:
                kts[p] = emit_k(p)
            if p not in vts:
                vts[p] = emit_v(
                    p, engine=nc.sync if p >= npairs - N_LATE_V else None
                )
            p_tile, l_part = emit_scores(p, kts[p], slice(0, ncht), ncht)
            emit_l_accum(p, l_part, stop=True)
            acc_p = psum_o.tile([1, hd], f32, tag="acc")
            emit_pv_start(p, acc_p)
            emit_pv(vts[p], acc_p, p_tile, 0, 0, ncht, stop_at_end=True)
            emit_normalize(p, acc_p)

        # ---- emit: one 16KB DMA of all normalized rows ----
        out_flat = bass.AP(
            tensor=out.tensor, offset=out.offset, ap=[[0, 1], [1, npairs * hd]]
        )
        nc.scalar.dma_start(out=out_flat, in_=final_row)

    nc.compile()
    return nc


@functools.lru_cache(maxsize=4)
def _program(bl, n_heads, max_seq, hd, cache_len):
    return _build_program(bl, n_heads, max_seq, hd, cache_len)


def kernel(query, key, value, key_cache, value_cache, cache_len):
    global LAST_RESULTS
    query = np.asarray(query, dtype=np.float32)
    key = np.asarray(key, dtype=np.float32)
    value = np.asarray(value, dtype=np.float32)
    key_cache = np.asarray(key_cache, dtype=np.float32)
    value_cache = np.asarray(value_cache, dtype=np.float32)
    cache_len = int(cache_len)

    b_sz, q_len, d_model = query.shape
    _, n_heads, max_seq, hd = key_cache.shape
    assert q_len == 1 and d_model == n_heads * hd
    assert b_sz % N_CORES == 0
    bl = b_sz // N_CORES

    prog = _program(bl, n_heads, max_seq, hd, cache_len)

    ident = np.eye(P, dtype=np.float32)
    in_maps = []
    for i in range(N_CORES):
        sl = slice(i * bl, (i + 1) * bl)
        in_maps.append(
            {
                "kc": np.ascontiguousarray(key_cache[sl]),
                "vc": np.ascontiguousarray(value_cache[sl]),
                "q": np.ascontiguousarray(query[sl]).reshape(bl * n_heads, hd),
                "kn": np.ascontiguousarray(key[sl]).reshape(bl * n_heads, hd),
                "vn": np.ascontiguousarray(value[sl]).reshape(bl * n_heads, hd),
                "ident": ident,
            }
        )

    try:
        res = bass_utils.run_bass_kernel_spmd(
            prog, in_maps, core_ids=list(range(N_CORES)), trace=TRACE, **TRACE_KWARGS
        )
    except Exception:
        # A previously crashed NeuronCore can leave the first execution
        # attempt failing with a transient runtime error; retry once.
        res = bass_utils.run_bass_kernel_spmd(
            prog, in_maps, core_ids=list(range(N_CORES)), trace=TRACE, **TRACE_KWARGS
        )
    LAST_RESULTS = res
    outs = [res.results[i]["out"].reshape(bl, q_len, d_model) for i in range(N_CORES)]
    return np.concatenate(outs, axis=0)
